# revision 2
# baseline (speedup 1.0000x reference)
"""Trainium2 Bass kernel for nn_Device_Policy (segment_reduce).

Strategy: shard the node axis N across 8 NeuronCores.  All large inputs
are pre-arranged host-side into the exact SBUF block layout and
quantized (mpnn -> bf16, mask/state -> fp8e4m3; 0/1 mask is exact in
fp8), cutting HBM traffic per core from 33.5 MB to 12.6 MB and removing
every PE transpose / int->float conversion from the device program.

Per core, for each block of 128 nodes (256 blocks):
  - dse^T[h, d] += mpnn_blk[k=node, h].T @ maskT_blk[k=node, d]
    (mpnn is the FWL-eligible bf16 stationary operand, fp8 mask streams)
  - state col sums / col sums-of-squares via ones-weight matmuls
    (squares produced per-chunk on the Scalar engine, fp8 -> bf16)
All three accumulate in fp32 PSUM across the whole loop.  The [128,65]
fp32 pack (dse^T | state stats) is AllGather'd (cheaper floor than
AllReduce), tree-summed on-device, and the tiny replicated MLP head
produces the [64] output on every core.
"""

import sys

if "/opt/trn_rl_repo" not in sys.path:
    sys.path.insert(0, "/opt/trn_rl_repo")

import ml_dtypes
import numpy as np

import concourse.bacc as bacc
import concourse.bass as bass
import concourse.mybir as mybir
import concourse.tile as tile
from concourse.bass_utils import run_bass_kernel_spmd

NCORES = 8
N = 262144
F = 64
D = 64
DF = 32
H1 = 128
H2 = 64
NSH = N // NCORES          # nodes per core = 32768
NBLK = NSH // 128          # 256 blocks of 128 nodes
NCHUNK = 8
BPC = NBLK // NCHUNK       # 32 blocks per chunk
EPS = 1e-6
SLOPE = 0.1

f32 = mybir.dt.float32
bf16 = mybir.dt.bfloat16
f8 = mybir.dt.float8e4
ADD = mybir.AluOpType.add
MUL = mybir.AluOpType.mult
SUB = mybir.AluOpType.subtract
BYPASS = mybir.AluOpType.bypass
AX = mybir.AxisListType.X
IDENT = mybir.ActivationFunctionType.Identity
SQUARE = mybir.ActivationFunctionType.Square
SQRT = mybir.ActivationFunctionType.Sqrt

NP_BF16 = ml_dtypes.bfloat16
NP_F8 = ml_dtypes.float8_e4m3fn


def build_program(stage="full"):
    nc = bacc.Bacc(
        "TRN2",
        target_bir_lowering=False,
        debug=False,
        enable_asserts=False,
        num_devices=NCORES,
    )

    x_mpnn = nc.dram_tensor("x_mpnn", [128, NBLK * H1], bf16, kind="ExternalInput")
    x_mask = nc.dram_tensor("x_mask", [128, NBLK * D], f8, kind="ExternalInput")
    x_state = nc.dram_tensor("x_state", [128, NBLK * F], f8, kind="ExternalInput")
    x_dfsT = nc.dram_tensor("x_dfsT", [64, D], f32, kind="ExternalInput")
    x_w1T = nc.dram_tensor("x_w1T", [64, H1], f32, kind="ExternalInput")
    x_b1 = nc.dram_tensor("x_b1", [H1, 1], f32, kind="ExternalInput")
    x_w2T = nc.dram_tensor("x_w2T", [F, H1], f32, kind="ExternalInput")
    x_b2 = nc.dram_tensor("x_b2", [H1, 1], f32, kind="ExternalInput")
    x_w3Tp = nc.dram_tensor("x_w3Tp", [H1, 4 * H2], f32, kind="ExternalInput")
    x_b3 = nc.dram_tensor("x_b3", [H2, 1], f32, kind="ExternalInput")
    x_w4T = nc.dram_tensor("x_w4T", [H2, 1], f32, kind="ExternalInput")
    x_b4 = nc.dram_tensor("x_b4", [D, 1], f32, kind="ExternalInput")
    x_spred = nc.dram_tensor("x_spred", [F, 1], f32, kind="ExternalInput")
    x_mpred = nc.dram_tensor("x_mpred", [H1, 1], f32, kind="ExternalInput")
    y_out = nc.dram_tensor("y_out", [D], f32, kind="ExternalOutput")
    y_dbg = None
    if stage != "full":
        y_dbg = nc.dram_tensor("y_dbg", [128, 65], f32, kind="ExternalOutput")

    with tile.TileContext(nc) as tc:
        emit(nc, tc, x_mpnn, x_mask, x_state, x_dfsT, x_w1T, x_b1, x_w2T,
             x_b2, x_w3Tp, x_b3, x_w4T, x_b4, x_spred, x_mpred, y_out,
             stage=stage, y_dbg=y_dbg)

    nc.compile()
    return nc


def emit(nc, tc, x_mpnn, x_mask, x_state, x_dfsT, x_w1T, x_b1, x_w2T, x_b2,
         x_w3Tp, x_b3, x_w4T, x_b4, x_spred, x_mpred, y_out,
         stage="full", y_dbg=None):
    ctx_pools = []

    def pool(name, bufs, space="SBUF"):
        p = tc.tile_pool(name=name, bufs=bufs, space=space)
        ctx_pools.append(p)
        return p.__enter__()

    cpool = pool("const", 1)
    mpnn_pool = pool("mpnn", 3)
    mask_pool = pool("mask", 3)
    st_pool = pool("st", 3)
    sq_pool = pool("sq", 2)
    ep_pool = pool("ep", 1)
    dse_psum = pool("dsepsum", 1, space="PSUM")
    s_psum = pool("spsum", 1, space="PSUM")
    ep_psum = pool("eppsum", 2, space="PSUM")
    dram_pool = pool("dram", 1, space="DRAM")

    # ---- constants ----
    ones = cpool.tile([128, 1], bf16, name="ones")
    nc.vector.memset(ones[:, :], 1.0)
    one1 = cpool.tile([1, 1], f32, name="one1")
    nc.vector.memset(one1[:, :], 1.0)
    zeros = cpool.tile([128, D], f32, name="zeros")
    nc.vector.memset(zeros[:, :], 0.0)

    dfsT = cpool.tile([64, D], f32, name="dfsT")
    nc.sync.dma_start(dfsT[:, :], x_dfsT[:, :])
    w1T = cpool.tile([64, H1], f32, name="w1T")
    nc.sync.dma_start(w1T[:, :], x_w1T[:, :])
    b1 = cpool.tile([H1, 1], f32, name="b1")
    nc.sync.dma_start(b1[:, :], x_b1[:, :])
    w2T = cpool.tile([F, H1], f32, name="w2T")
    nc.sync.dma_start(w2T[:, :], x_w2T[:, :])
    b2 = cpool.tile([H1, 1], f32, name="b2")
    nc.sync.dma_start(b2[:, :], x_b2[:, :])
    w3Tp = cpool.tile([H1, 4 * H2], f32, name="w3Tp")
    nc.sync.dma_start(w3Tp[:, :], x_w3Tp[:, :])
    b3 = cpool.tile([H2, 1], f32, name="b3")
    nc.sync.dma_start(b3[:, :], x_b3[:, :])
    w4T = cpool.tile([H2, 1], f32, name="w4T")
    nc.sync.dma_start(w4T[:, :], x_w4T[:, :])
    b4 = cpool.tile([D, 1], f32, name="b4")
    nc.sync.dma_start(b4[:, :], x_b4[:, :])
    spred = cpool.tile([F, 1], f32, name="spred")
    nc.sync.dma_start(spred[:, :], x_spred[:, :])
    mpred = cpool.tile([H1, 1], f32, name="mpred")
    nc.sync.dma_start(mpred[:, :], x_mpred[:, :])

    # persistent PSUM accumulators for the whole node loop
    psum_dse = dse_psum.tile([H1, D], f32, name="psum_dse", tag="psum_dse")
    psum_s = s_psum.tile([1, F], f32, name="psum_s", tag="psum_s")
    psum_q = s_psum.tile([1, F], f32, name="psum_q", tag="psum_q")

    # ---- main loop: stream chunks of 32 blocks (4096 nodes) ----
    for t in range(NCHUNK):
        mp = mpnn_pool.tile([128, BPC * H1], bf16, name="mp", tag="mp")
        nc.sync.dma_start(mp[:, :],
                          x_mpnn[:, t * BPC * H1:(t + 1) * BPC * H1])
        mk = mask_pool.tile([128, BPC * D], f8, name="mk", tag="mk")
        nc.sync.dma_start(mk[:, :],
                          x_mask[:, t * BPC * D:(t + 1) * BPC * D])
        st = st_pool.tile([128, BPC * F], f8, name="st", tag="st")
        nc.sync.dma_start(st[:, :],
                          x_state[:, t * BPC * F:(t + 1) * BPC * F])
        sq = sq_pool.tile([128, BPC * F], bf16, name="sq", tag="sq")
        nc.scalar.activation(sq[:, :], st[:, :], SQUARE)

        for b in range(BPC):
            g = t * BPC + b
            first, last = g == 0, g == NBLK - 1
            nc.tensor.matmul(
                psum_dse[:, :],
                lhsT=mp[:, b * H1:(b + 1) * H1],
                rhs=mk[:, b * D:(b + 1) * D],
                start=first, stop=last, skip_group_check=True,
            )
            nc.tensor.matmul(
                psum_s[:, :],
                lhsT=ones[:, :],
                rhs=st[:, b * F:(b + 1) * F],
                start=first, stop=last, skip_group_check=True,
            )
            nc.tensor.matmul(
                psum_q[:, :],
                lhsT=ones[:, :],
                rhs=sq[:, b * F:(b + 1) * F],
                start=first, stop=last, skip_group_check=True,
            )

    # ---- pack partials: [128, 65] = [dse^T | state sum / sumsq col] ----
    srow = ep_pool.tile([1, 2 * F], f32, name="srow", tag="srow")
    nc.vector.tensor_copy(srow[:, 0:F], psum_s[:, :])
    nc.vector.tensor_copy(srow[:, F:2 * F], psum_q[:, :])
    psum_t = ep_psum.tile([128, 1], f32, name="psum_t", tag="ep")
    nc.tensor.matmul(psum_t[:, :], lhsT=srow[:, :], rhs=one1[:, :],
                     start=True, stop=True)

    pack = ep_pool.tile([128, 65], f32, name="pack", tag="pack")
    nc.vector.tensor_copy(pack[:, 0:64], psum_dse[:, :])
    nc.vector.tensor_copy(pack[:, 64:65], psum_t[:, :])

    if stage == "loop":
        nc.sync.dma_start(y_dbg[:, :], pack[:, :])
        nc.sync.dma_start(y_out[:], pack[0, 0:64])
        for p in reversed(ctx_pools):
            p.__exit__(None, None, None)
        return

    # ---- AllGather partials (cheaper floor than AllReduce), sum on DVE ----
    cc_in = dram_pool.tile([128, 65], f32, name="cc_in", tag="cc_in")
    cc_out = dram_pool.tile([NCORES * 128, 65], f32, name="cc_out",
                            tag="cc_out", addr_space="Shared")
    nc.sync.dma_start(cc_in[:, :], pack[:, :])
    nc.gpsimd.collective_compute(
        "AllGather",
        BYPASS,
        replica_groups=[list(range(NCORES))],
        ins=[cc_in[:, :].opt()],
        outs=[cc_out[:, :].opt()],
    )
    red8 = ep_pool.tile([128, 8 * 65], f32, name="red8", tag="red8")
    nc.sync.dma_start(
        red8[:, :].rearrange("p (r c) -> p r c", r=8),
        cc_out[:, :].rearrange("(r p) c -> p r c", p=128),
    )
    t1 = ep_pool.tile([128, 4 * 65], f32, name="t1r", tag="t1r")
    nc.vector.tensor_add(t1[:, :], red8[:, 0:260], red8[:, 260:520])
    t2 = ep_pool.tile([128, 2 * 65], f32, name="t2r", tag="t2r")
    nc.vector.tensor_add(t2[:, :], t1[:, 0:130], t1[:, 130:260])
    red = ep_pool.tile([128, 65], f32, name="red", tag="red")
    nc.vector.tensor_add(red[:, :], t2[:, 0:65], t2[:, 65:130])

    if stage == "pack":
        nc.sync.dma_start(y_dbg[:, :], red[:, :])
        nc.sync.dma_start(y_out[:], red[0, 0:64])
        for p in reversed(ctx_pools):
            p.__exit__(None, None, None)
        return

    # ---- replicated MLP head ----
    dseT = red[:, 0:64]          # [128 h1, 64 d] global masked sums
    ssum = red[0:F, 64:65]       # [64 f, 1] global state column sums
    ssq = red[F:128, 64:65]      # [64 f, 1] global state column sum-squares

    # state per-feature mean / 1/(std+eps), as [F,1] columns
    mean_s = ep_pool.tile([F, 1], f32, name="mean_s", tag="mean_s")
    nc.vector.tensor_scalar_mul(mean_s[:, :], ssum, 1.0 / N)
    ex2_s = ep_pool.tile([F, 1], f32, name="ex2_s", tag="ex2_s")
    nc.vector.tensor_scalar_mul(ex2_s[:, :], ssq, 1.0 / N)
    var_s = ep_pool.tile([F, 1], f32, name="var_s", tag="var_s")
    nc.vector.tensor_mul(var_s[:, :], mean_s[:, :], mean_s[:, :])
    nc.vector.tensor_sub(var_s[:, :], ex2_s[:, :], var_s[:, :])
    std_s = ep_pool.tile([F, 1], f32, name="std_s", tag="std_s")
    nc.scalar.activation(std_s[:, :], var_s[:, :], SQRT)
    nc.vector.tensor_scalar_add(std_s[:, :], std_s[:, :], EPS)
    inv_s = ep_pool.tile([F, 1], f32, name="inv_s", tag="inv_s")
    nc.vector.reciprocal(inv_s[:, :], std_s[:, :])

    # rep_latent column: leaky(W2 @ xn + b2), then broadcast along D
    xn = ep_pool.tile([F, 1], f32, name="xn", tag="xn")
    nc.vector.tensor_scalar(xn[:, :], spred[:, :], mean_s[:, :], inv_s[:, :],
                            op0=SUB, op1=MUL)
    psum_rl = ep_psum.tile([H1, 1], f32, name="psum_rl", tag="ep")
    nc.tensor.matmul(psum_rl[:, :], lhsT=w2T[:, :], rhs=xn[:, :],
                     start=True, stop=True)
    rl = ep_pool.tile([H1, 1], f32, name="rl", tag="rl")
    nc.scalar.activation(rl[:, :], psum_rl[:, :], IDENT, bias=b2[:, :])
    rl_a = ep_pool.tile([H1, 1], f32, name="rl_a", tag="rl_a")
    nc.vector.tensor_scalar_mul(rl_a[:, :], rl[:, :], SLOPE)
    nc.vector.tensor_max(rl[:, :], rl[:, :], rl_a[:, :])
    rl_b = ep_pool.tile([H1, D], f32, name="rl_b", tag="rl_b")
    nc.scalar.activation(rl_b[:, :], zeros[:, :], IDENT, bias=rl[:, :])

    # device_feat_state normalization (over D, free axis) + embedding.
    # dfsT/w1T are zero-padded from 32 to 64 partitions host-side.
    # Depends only on inputs -> scheduled during the stream.
    mean_f = ep_pool.tile([64, 1], f32, name="mean_f", tag="mean_f")
    nc.vector.tensor_reduce(mean_f[:, :], dfsT[:, :], axis=AX, op=ADD)
    nc.vector.tensor_scalar_mul(mean_f[:, :], mean_f[:, :], 1.0 / D)
    sqf = ep_pool.tile([64, D], f32, name="sqf", tag="sqf")
    nc.scalar.activation(sqf[:, :], dfsT[:, :], SQUARE)
    qf = ep_pool.tile([64, 1], f32, name="qf", tag="qf")
    nc.vector.tensor_reduce(qf[:, :], sqf[:, :], axis=AX, op=ADD)
    nc.vector.tensor_scalar_mul(qf[:, :], qf[:, :], 1.0 / D)
    varf = ep_pool.tile([64, 1], f32, name="varf", tag="varf")
    nc.vector.tensor_mul(varf[:, :], mean_f[:, :], mean_f[:, :])
    nc.vector.tensor_sub(varf[:, :], qf[:, :], varf[:, :])
    stdf = ep_pool.tile([64, 1], f32, name="stdf", tag="stdf")
    nc.scalar.activation(stdf[:, :], varf[:, :], SQRT)
    nc.vector.tensor_scalar_add(stdf[:, :], stdf[:, :], EPS)
    invf = ep_pool.tile([64, 1], f32, name="invf", tag="invf")
    nc.vector.reciprocal(invf[:, :], stdf[:, :])
    dfsn = ep_pool.tile([64, D], f32, name="dfsn", tag="dfsn")
    nc.vector.tensor_scalar(dfsn[:, :], dfsT[:, :], mean_f[:, :], invf[:, :],
                            op0=SUB, op1=MUL)
    psum_dfe = ep_psum.tile([H1, D], f32, name="psum_dfe", tag="ep")
    nc.tensor.matmul(psum_dfe[:, :], lhsT=w1T[:, :], rhs=dfsn[:, :],
                     start=True, stop=True)
    dfeT = ep_pool.tile([H1, D], f32, name="dfeT", tag="dfeT")
    nc.scalar.activation(dfeT[:, :], psum_dfe[:, :], IDENT, bias=b1[:, :])
    dfe_a = ep_pool.tile([H1, D], f32, name="dfe_a", tag="dfe_a")
    nc.vector.tensor_scalar_mul(dfe_a[:, :], dfeT[:, :], SLOPE)
    nc.vector.tensor_max(dfeT[:, :], dfeT[:, :], dfe_a[:, :])

    # dse normalization (over D, free axis)
    mean_d = ep_pool.tile([H1, 1], f32, name="mean_d", tag="mean_d")
    nc.vector.tensor_reduce(mean_d[:, :], dseT, axis=AX, op=ADD)
    nc.vector.tensor_scalar_mul(mean_d[:, :], mean_d[:, :], 1.0 / D)
    sqd = ep_pool.tile([H1, D], f32, name="sqd", tag="sqd")
    nc.scalar.activation(sqd[:, :], dseT, SQUARE)
    qd = ep_pool.tile([H1, 1], f32, name="qd", tag="qd")
    nc.vector.tensor_reduce(qd[:, :], sqd[:, :], axis=AX, op=ADD)
    nc.vector.tensor_scalar_mul(qd[:, :], qd[:, :], 1.0 / D)
    vard = ep_pool.tile([H1, 1], f32, name="vard", tag="vard")
    nc.vector.tensor_mul(vard[:, :], mean_d[:, :], mean_d[:, :])
    nc.vector.tensor_sub(vard[:, :], qd[:, :], vard[:, :])
    stdd = ep_pool.tile([H1, 1], f32, name="stdd", tag="stdd")
    nc.scalar.activation(stdd[:, :], vard[:, :], SQRT)
    nc.vector.tensor_scalar_add(stdd[:, :], stdd[:, :], EPS)
    invd = ep_pool.tile([H1, 1], f32, name="invd", tag="invd")
    nc.vector.reciprocal(invd[:, :], stdd[:, :])
    dsen = ep_pool.tile([H1, D], f32, name="dsen", tag="dsen")
    nc.vector.tensor_scalar(dsen[:, :], dseT, mean_d[:, :], invd[:, :],
                            op0=SUB, op1=MUL)

    # broadcast mpnn[pred] along the D axis (input-only -> runs early)
    repe = ep_pool.tile([H1, D], f32, name="repe", tag="repe")
    nc.scalar.activation(repe[:, :], zeros[:, :], IDENT, bias=mpred[:, :])

    # h.T = leaky(W3 @ concat.T + b3): 4 accumulated chunks over c=512
    psum_h = ep_psum.tile([H2, D], f32, name="psum_h", tag="ep")
    chunks = [dfeT[:, :], rl_b[:, :], repe[:, :], dsen[:, :]]
    for k in range(4):
        nc.tensor.matmul(psum_h[:, :], lhsT=w3Tp[:, k * H2:(k + 1) * H2],
                         rhs=chunks[k], start=(k == 0), stop=(k == 3))
    hT = ep_pool.tile([H2, D], f32, name="hT", tag="hT")
    nc.scalar.activation(hT[:, :], psum_h[:, :], IDENT, bias=b3[:, :])
    hT_a = ep_pool.tile([H2, D], f32, name="hT_a", tag="hT_a")
    nc.vector.tensor_scalar_mul(hT_a[:, :], hT[:, :], SLOPE)
    nc.vector.tensor_max(hT[:, :], hT[:, :], hT_a[:, :])

    # output[d] = sum_j hT[j, d] * W4[0, j] + b4
    psum_o = ep_psum.tile([D, 1], f32, name="psum_o", tag="ep")
    nc.tensor.matmul(psum_o[:, :], lhsT=hT[:, :], rhs=w4T[:, :],
                     start=True, stop=True)
    out_sb = ep_pool.tile([D, 1], f32, name="out_sb", tag="out_sb")
    nc.scalar.activation(out_sb[:, :], psum_o[:, :], IDENT, bias=b4[:, :])
    nc.sync.dma_start(y_out[:], out_sb[:, 0])

    for p in reversed(ctx_pools):
        p.__exit__(None, None, None)


_compiled = None


def _get_compiled():
    global _compiled
    if _compiled is None:
        _compiled = build_program()
    return _compiled


def make_in_maps(inputs):
    state = np.asarray(inputs["state"], dtype=np.float32)
    dfs = np.asarray(inputs["device_feat_state"], dtype=np.float32)
    mpnn = np.asarray(inputs["mpnn_forward"], dtype=np.float32)
    W1 = np.asarray(inputs["W1"], dtype=np.float32)
    b1 = np.asarray(inputs["b1"], dtype=np.float32)
    W2 = np.asarray(inputs["W2"], dtype=np.float32)
    b2 = np.asarray(inputs["b2"], dtype=np.float32)
    W3 = np.asarray(inputs["W3"], dtype=np.float32)
    b3 = np.asarray(inputs["b3"], dtype=np.float32)
    W4 = np.asarray(inputs["W4"], dtype=np.float32)
    b4 = np.asarray(inputs["b4"], dtype=np.float32)
    mask = np.asarray(inputs["device_assign_state"])
    pred = int(np.asarray(inputs["pred_node"]))

    # block layouts: element (p, b*W + j) = full[b*128 + p, j]
    mp_l = mpnn.astype(NP_BF16).reshape(N // 128, 128, H1).transpose(1, 0, 2)
    st_l = state.astype(NP_F8).reshape(N // 128, 128, F).transpose(1, 0, 2)
    # 0/1 mask -> exact fp8e4m3 via the 0x38 (==1.0) byte trick
    mk8 = ((mask == 1).astype(np.uint8) * np.uint8(0x38)).view(NP_F8)
    mk_l = np.ascontiguousarray(mk8.reshape(D, N // 128, 128).transpose(1, 2, 0))
    # mk_l[b, p, d] = mask[d, b*128+p]

    w3Tp = np.ascontiguousarray(
        W3.T.reshape(4, H1, H2).transpose(1, 0, 2).reshape(H1, 4 * H2))
    common = {
        "x_dfsT": np.ascontiguousarray(np.pad(dfs.T, ((0, 64 - DF), (0, 0)))),
        "x_w1T": np.ascontiguousarray(np.pad(W1.T, ((0, 64 - DF), (0, 0)))),
        "x_b1": np.ascontiguousarray(b1.reshape(H1, 1)),
        "x_w2T": np.ascontiguousarray(W2.T),
        "x_b2": np.ascontiguousarray(b2.reshape(H1, 1)),
        "x_w3Tp": w3Tp,
        "x_b3": np.ascontiguousarray(b3.reshape(H2, 1)),
        "x_w4T": np.ascontiguousarray(W4.T),
        "x_b4": np.ascontiguousarray(np.broadcast_to(b4.reshape(1, 1), (D, 1))),
        "x_spred": np.ascontiguousarray(state[pred].reshape(F, 1)),
        "x_mpred": np.ascontiguousarray(mpnn[pred].reshape(H1, 1)),
    }
    in_maps = []
    for c in range(NCORES):
        bs = slice(c * NBLK, (c + 1) * NBLK)
        in_maps.append({
            **common,
            "x_mpnn": np.ascontiguousarray(mp_l[:, bs, :]).reshape(128, NBLK * H1),
            "x_state": np.ascontiguousarray(st_l[:, bs, :]).reshape(128, NBLK * F),
            "x_mask": np.ascontiguousarray(
                mk_l[bs].transpose(1, 0, 2)).reshape(128, NBLK * D),
        })
    return in_maps


def kernel(**inputs) -> np.ndarray:
    nc = _get_compiled()
    in_maps = make_in_maps(inputs)
    res = run_bass_kernel_spmd(nc, in_maps, core_ids=list(range(NCORES)))
    return np.asarray(res.results[0]["y_out"], dtype=np.float32)


# revision 5
# speedup vs baseline: 1.0620x; 1.0620x over previous
"""Trainium2 Bass kernel for nn_Device_Policy (segment_reduce).

Strategy: shard the node axis N across 8 NeuronCores.  All large inputs
are pre-arranged host-side into the exact SBUF block layout and
quantized (mpnn -> bf16, mask/state -> fp8e4m3; the 0/1 mask is exact
in fp8), cutting HBM traffic per core from 33.5 MB to 12.6 MB and
removing every PE transpose / int->float conversion on device.

Per core:
  - dse^T[h, d] += mpnn_blk[node, h].T @ maskT_blk[node, d] per
    128-node block (FWL bf16 stationary mpnn, fp8 mask streams),
    accumulated in fp32 PSUM across all 256 blocks.
  - state col sums / sums-of-squares as ones-weight matmuls with
    N=512 moving slices (squares via ScalarE, fp8 -> bf16).
  - state stats finish early and go through a small AllReduce that
    completes while mpnn is still streaming; the dependent part of the
    MLP head (pred-node latent path) also runs during the stream.
  - the [128, 64] dse^T partial is AllGather'd (lower floor than
    AllReduce), tree-summed on DVE, and only the dse-normalization +
    final two matmuls remain in the exposed tail.
"""

import sys

if "/opt/trn_rl_repo" not in sys.path:
    sys.path.insert(0, "/opt/trn_rl_repo")

import ml_dtypes
import numpy as np

import concourse.bacc as bacc
import concourse.bass as bass
import concourse.mybir as mybir
import concourse.tile as tile
from concourse.bass_utils import run_bass_kernel_spmd

NCORES = 8
N = 262144
F = 64
D = 64
DF = 32
H1 = 128
H2 = 64
NSH = N // NCORES          # nodes per core = 32768
NBLK = NSH // 128          # 256 blocks of 128 nodes
NCHUNK = 8
BPC = NBLK // NCHUNK       # 32 blocks per mpnn chunk
NSLICE = NBLK * F // 512   # 32 N=512 state slices
EPS = 1e-6
SLOPE = 0.1

f32 = mybir.dt.float32
bf16 = mybir.dt.bfloat16
f8 = mybir.dt.float8e4
ADD = mybir.AluOpType.add
MUL = mybir.AluOpType.mult
SUB = mybir.AluOpType.subtract
BYPASS = mybir.AluOpType.bypass
AX = mybir.AxisListType.X
IDENT = mybir.ActivationFunctionType.Identity
SQUARE = mybir.ActivationFunctionType.Square
SQRT = mybir.ActivationFunctionType.Sqrt

NP_BF16 = ml_dtypes.bfloat16
NP_F8 = ml_dtypes.float8_e4m3fn


def build_program(stage="full"):
    nc = bacc.Bacc(
        "TRN2",
        target_bir_lowering=False,
        debug=False,
        enable_asserts=False,
        num_devices=NCORES,
    )

    x_mpnn = nc.dram_tensor("x_mpnn", [128, NBLK * H1], bf16, kind="ExternalInput")
    x_mask = nc.dram_tensor("x_mask", [128, NBLK * D], f8, kind="ExternalInput")
    x_state = nc.dram_tensor("x_state", [128, NBLK * F], f8, kind="ExternalInput")
    x_dfsT = nc.dram_tensor("x_dfsT", [64, D], f32, kind="ExternalInput")
    x_w1T = nc.dram_tensor("x_w1T", [64, H1], f32, kind="ExternalInput")
    x_b1 = nc.dram_tensor("x_b1", [H1, 1], f32, kind="ExternalInput")
    x_w2T = nc.dram_tensor("x_w2T", [F, H1], f32, kind="ExternalInput")
    x_b2 = nc.dram_tensor("x_b2", [H1, 1], f32, kind="ExternalInput")
    x_w3Tp = nc.dram_tensor("x_w3Tp", [H1, 4 * H2], f32, kind="ExternalInput")
    x_b3 = nc.dram_tensor("x_b3", [H2, 1], f32, kind="ExternalInput")
    x_w4T = nc.dram_tensor("x_w4T", [H2, 1], f32, kind="ExternalInput")
    x_b4 = nc.dram_tensor("x_b4", [D, 1], f32, kind="ExternalInput")
    x_spred = nc.dram_tensor("x_spred", [F, 1], f32, kind="ExternalInput")
    x_mpred = nc.dram_tensor("x_mpred", [H1, 1], f32, kind="ExternalInput")
    y_out = nc.dram_tensor("y_out", [D], f32, kind="ExternalOutput")
    y_dbg = None
    if stage != "full":
        y_dbg = nc.dram_tensor("y_dbg", [128, 65], f32, kind="ExternalOutput")

    with tile.TileContext(nc) as tc:
        emit(nc, tc, x_mpnn, x_mask, x_state, x_dfsT, x_w1T, x_b1, x_w2T,
             x_b2, x_w3Tp, x_b3, x_w4T, x_b4, x_spred, x_mpred, y_out,
             stage=stage, y_dbg=y_dbg)

    nc.compile()
    return nc


def emit(nc, tc, x_mpnn, x_mask, x_state, x_dfsT, x_w1T, x_b1, x_w2T, x_b2,
         x_w3Tp, x_b3, x_w4T, x_b4, x_spred, x_mpred, y_out,
         stage="full", y_dbg=None):
    ctx_pools = []

    def pool(name, bufs, space="SBUF"):
        p = tc.tile_pool(name=name, bufs=bufs, space=space)
        ctx_pools.append(p)
        return p.__enter__()

    cpool = pool("const", 1)
    mpnn_pool = pool("mpnn", 3)
    mk_pool = pool("mk", 1)
    st_pool = pool("stq", 1)
    sq_pool = pool("sq", 1)
    ep_pool = pool("ep", 1)
    dse_psum = pool("dsepsum", 1, space="PSUM")
    s_psum = pool("spsum", 1, space="PSUM")
    ep_psum = pool("eppsum", 2, space="PSUM")
    dram_pool = pool("dram", 1, space="DRAM")

    # ---- SBUF constants via memset (no DMA) ----
    ones = cpool.tile([128, 1], bf16, name="ones")
    nc.vector.memset(ones[:, :], 1.0)
    one1 = cpool.tile([1, 1], f32, name="one1")
    nc.vector.memset(one1[:, :], 1.0)
    zeros = cpool.tile([128, D], f32, name="zeros")
    nc.vector.memset(zeros[:, :], 0.0)

    # ---- big stream DMAs issued first (sync queue) ----
    # state quarters first (their stats feed the early AllReduce)
    st_q = []
    for q in range(4):
        t_ = st_pool.tile([128, NBLK * F // 4], f8, name=f"stq{q}", tag=f"stq{q}")
        nc.sync.dma_start(t_[:, :],
                          x_state[:, q * NBLK * F // 4:(q + 1) * NBLK * F // 4])
        st_q.append(t_)
    mk_h = []
    for hh in range(2):
        t_ = mk_pool.tile([128, NBLK * D // 2], f8, name=f"mkh{hh}", tag=f"mkh{hh}")
        nc.sync.dma_start(t_[:, :],
                          x_mask[:, hh * NBLK * D // 2:(hh + 1) * NBLK * D // 2])
        mk_h.append(t_)

    # ---- small constants on the scalar HWDGE queue (overlap stream) ----
    def cload(name, shape, src):
        t_ = cpool.tile(shape, f32, name=name)
        nc.scalar.dma_start(t_[:, :], src[:, :])
        return t_

    dfsT = cload("dfsT", [64, D], x_dfsT)
    w1T = cload("w1T", [64, H1], x_w1T)
    b1 = cload("b1", [H1, 1], x_b1)
    w2T = cload("w2T", [F, H1], x_w2T)
    b2 = cload("b2", [H1, 1], x_b2)
    w3Tp = cload("w3Tp", [H1, 4 * H2], x_w3Tp)
    b3 = cload("b3", [H2, 1], x_b3)
    w4T = cload("w4T", [H2, 1], x_w4T)
    b4 = cload("b4", [D, 1], x_b4)
    spred = cload("spred", [F, 1], x_spred)
    mpred = cload("mpred", [H1, 1], x_mpred)

    # persistent PSUM accumulators
    psum_dse = dse_psum.tile([H1, D], f32, name="psum_dse", tag="psum_dse")
    psum_s = s_psum.tile([1, 512], f32, name="psum_s", tag="psum_s")
    psum_q = s_psum.tile([1, 512], f32, name="psum_q", tag="psum_q")

    # ---- constants-only epilogue pieces (run during the stream) ----
    # device_feat embedding: normalize over D (free axis) + W1 + leaky.
    # dfsT/w1T zero-padded 32 -> 64 partitions host-side.
    mean_f = ep_pool.tile([64, 1], f32, name="mean_f", tag="mean_f")
    nc.vector.tensor_reduce(mean_f[:, :], dfsT[:, :], axis=AX, op=ADD)
    nc.vector.tensor_scalar_mul(mean_f[:, :], mean_f[:, :], 1.0 / D)
    sqf = ep_pool.tile([64, D], f32, name="sqf", tag="sqf")
    nc.scalar.activation(sqf[:, :], dfsT[:, :], SQUARE)
    qf = ep_pool.tile([64, 1], f32, name="qf", tag="qf")
    nc.vector.tensor_reduce(qf[:, :], sqf[:, :], axis=AX, op=ADD)
    varf = ep_pool.tile([64, 1], f32, name="varf", tag="varf")
    nc.vector.tensor_mul(varf[:, :], mean_f[:, :], mean_f[:, :])
    nc.vector.tensor_scalar(varf[:, :], qf[:, :], 1.0 / D, varf[:, :],
                            op0=MUL, op1=SUB)
    stdf = ep_pool.tile([64, 1], f32, name="stdf", tag="stdf")
    nc.scalar.activation(stdf[:, :], varf[:, :], SQRT)
    nc.vector.tensor_scalar_add(stdf[:, :], stdf[:, :], EPS)
    invf = ep_pool.tile([64, 1], f32, name="invf", tag="invf")
    nc.vector.reciprocal(invf[:, :], stdf[:, :])
    dfsn = ep_pool.tile([64, D], f32, name="dfsn", tag="dfsn")
    nc.vector.tensor_scalar(dfsn[:, :], dfsT[:, :], mean_f[:, :], invf[:, :],
                            op0=SUB, op1=MUL)
    repe = ep_pool.tile([H1, D], f32, name="repe", tag="repe")
    nc.scalar.activation(repe[:, :], zeros[:, :], IDENT, bias=mpred[:, :])

    # ---- main loop: mpnn chunks + dse matmuls; state work on chunks 0-3 ----
    for t in range(NCHUNK):
        mp = mpnn_pool.tile([128, BPC * H1], bf16, name="mp", tag="mp")
        nc.sync.dma_start(mp[:, :],
                          x_mpnn[:, t * BPC * H1:(t + 1) * BPC * H1])
        if t < 4:
            # squares for state quarter t (fp8 -> bf16 on ScalarE)
            sq = sq_pool.tile([128, NBLK * F // 4], bf16, name=f"sq{t}",
                              tag=f"sq{t}")
            nc.scalar.activation(sq[:, :], st_q[t][:, :], SQUARE)
        for b in range(BPC):
            g = t * BPC + b
            mh = mk_h[g // (NBLK // 2)]
            off = (g % (NBLK // 2)) * D
            nc.tensor.matmul(
                psum_dse[:, :],
                lhsT=mp[:, b * H1:(b + 1) * H1],
                rhs=mh[:, off:off + D],
                start=(g == 0), stop=(g == NBLK - 1), skip_group_check=True,
            )
        if t < 4:
            # 8 st + 8 sq matmuls, N=512 each, vs the ones weight
            for j in range(8):
                s0 = j * 512
                nc.tensor.matmul(
                    psum_s[:, :], lhsT=ones[:, :],
                    rhs=st_q[t][:, s0:s0 + 512],
                    start=(t == 0 and j == 0), stop=(t == 3 and j == 7),
                    skip_group_check=True,
                )
            for j in range(8):
                s0 = j * 512
                nc.tensor.matmul(
                    psum_q[:, :], lhsT=ones[:, :],
                    rhs=sq[:, s0:s0 + 512],
                    start=(t == 0 and j == 0), stop=(t == 3 and j == 7),
                    skip_group_check=True,
                )
        if t == 3:
            # state sums done: fold [1,512] -> [1,64], transpose to a
            # column through the PE, AllReduce it (hidden by the stream)
            srow = ep_pool.tile([1, 2 * F], f32, name="srow", tag="srow")
            for which, ps in (("s", psum_s), ("q", psum_q)):
                u0 = ep_pool.tile([1, 512], f32, name=f"u0{which}", tag=f"u0{which}")
                nc.vector.tensor_copy(u0[:, :], ps[0:1, :])
                u1 = ep_pool.tile([1, 256], f32, name=f"u1{which}", tag=f"u1{which}")
                nc.vector.tensor_add(u1[:, :], u0[:, 0:256], u0[:, 256:512])
                u2 = ep_pool.tile([1, 128], f32, name=f"u2{which}", tag=f"u2{which}")
                nc.vector.tensor_add(u2[:, :], u1[:, 0:128], u1[:, 128:256])
                dst = srow[0:1, 0:F] if which == "s" else srow[0:1, F:2 * F]
                nc.vector.tensor_add(dst, u2[:, 0:64], u2[:, 64:128])
            psum_t = ep_psum.tile([128, 1], f32, name="psum_t", tag="ep")
            nc.tensor.matmul(psum_t[:, :], lhsT=srow[:, :], rhs=one1[:, :],
                             start=True, stop=True)
            scol = ep_pool.tile([128, 1], f32, name="scol", tag="scol")
            nc.vector.tensor_copy(scol[:, :], psum_t[:, :])
            ar_in = dram_pool.tile([128, 1], f32, name="ar_in", tag="ar_in")
            ar_out = dram_pool.tile([128, 1], f32, name="ar_out", tag="ar_out",
                                    addr_space="Shared")
            nc.sync.dma_start(ar_in[:, :], scol[:, :])
            nc.gpsimd.collective_compute(
                "AllReduce", ADD,
                replica_groups=[list(range(NCORES))],
                ins=[ar_in[:, :].opt()],
                outs=[ar_out[:, :].opt()],
            )
            gstats = ep_pool.tile([128, 1], f32, name="gstats", tag="gstats")
            nc.scalar.dma_start(gstats[:, :], ar_out[:, :])

            # global state stats -> normalized pred row -> rep-latent col
            ssum = gstats[0:F, 0:1]
            ssq = gstats[F:128, 0:1]
            mean_s = ep_pool.tile([F, 1], f32, name="mean_s", tag="mean_s")
            nc.vector.tensor_scalar_mul(mean_s[:, :], ssum, 1.0 / N)
            var_s = ep_pool.tile([F, 1], f32, name="var_s", tag="var_s")
            nc.vector.tensor_mul(var_s[:, :], mean_s[:, :], mean_s[:, :])
            nc.vector.tensor_scalar(var_s[:, :], ssq, 1.0 / N, var_s[:, :],
                                    op0=MUL, op1=SUB)
            std_s = ep_pool.tile([F, 1], f32, name="std_s", tag="std_s")
            nc.scalar.activation(std_s[:, :], var_s[:, :], SQRT)
            nc.vector.tensor_scalar_add(std_s[:, :], std_s[:, :], EPS)
            inv_s = ep_pool.tile([F, 1], f32, name="inv_s", tag="inv_s")
            nc.vector.reciprocal(inv_s[:, :], std_s[:, :])
            xn = ep_pool.tile([F, 1], f32, name="xn", tag="xn")
            nc.vector.tensor_scalar(xn[:, :], spred[:, :], mean_s[:, :],
                                    inv_s[:, :], op0=SUB, op1=MUL)

    # rep-latent: leaky(W2 @ xn + b2), broadcast along D (pre-AG work;
    # PE reaches these after the dse matmuls, waits only on xn)
    psum_rl = ep_psum.tile([H1, 1], f32, name="psum_rl", tag="ep")
    nc.tensor.matmul(psum_rl[:, :], lhsT=w2T[:, :], rhs=xn[:, :],
                     start=True, stop=True)
    rl = ep_pool.tile([H1, 1], f32, name="rl", tag="rl")
    nc.scalar.activation(rl[:, :], psum_rl[:, :], IDENT, bias=b2[:, :])
    rl_a = ep_pool.tile([H1, 1], f32, name="rl_a", tag="rl_a")
    nc.vector.tensor_scalar_mul(rl_a[:, :], rl[:, :], SLOPE)
    nc.vector.tensor_max(rl[:, :], rl[:, :], rl_a[:, :])
    rl_b = ep_pool.tile([H1, D], f32, name="rl_b", tag="rl_b")
    nc.scalar.activation(rl_b[:, :], zeros[:, :], IDENT, bias=rl[:, :])

    # dfe embedding matmul + leaky (constants-ready, runs pre-AG)
    psum_dfe = ep_psum.tile([H1, D], f32, name="psum_dfe", tag="ep")
    nc.tensor.matmul(psum_dfe[:, :], lhsT=w1T[:, :], rhs=dfsn[:, :],
                     start=True, stop=True)
    dfeT = ep_pool.tile([H1, D], f32, name="dfeT", tag="dfeT")
    nc.scalar.activation(dfeT[:, :], psum_dfe[:, :], IDENT, bias=b1[:, :])
    dfe_a = ep_pool.tile([H1, D], f32, name="dfe_a", tag="dfe_a")
    nc.vector.tensor_scalar_mul(dfe_a[:, :], dfeT[:, :], SLOPE)
    nc.vector.tensor_max(dfeT[:, :], dfeT[:, :], dfe_a[:, :])

    # ---- pack dse partial and AllGather it ----
    pack = ep_pool.tile([128, 64], f32, name="pack", tag="pack")
    nc.vector.tensor_copy(pack[:, :], psum_dse[:, :])

    if stage == "loop":
        nc.sync.dma_start(y_dbg[:, 0:64], pack[:, :])
        nc.sync.dma_start(y_dbg[:, 64:65], scol[:, :])
        nc.sync.dma_start(y_out[:], pack[0, 0:64])
        for p in reversed(ctx_pools):
            p.__exit__(None, None, None)
        return

    cc_in = dram_pool.tile([128, 64], f32, name="cc_in", tag="cc_in")
    cc_out = dram_pool.tile([NCORES * 128, 64], f32, name="cc_out",
                            tag="cc_out", addr_space="Shared")
    nc.sync.dma_start(cc_in[:, :], pack[:, :])
    nc.gpsimd.collective_compute(
        "AllGather", BYPASS,
        replica_groups=[list(range(NCORES))],
        ins=[cc_in[:, :].opt()],
        outs=[cc_out[:, :].opt()],
    )
    red8 = ep_pool.tile([128, 8 * 64], f32, name="red8", tag="red8")
    nc.sync.dma_start(
        red8[:, :].rearrange("p (r c) -> p r c", r=8),
        cc_out[:, :].rearrange("(r p) c -> p r c", p=128),
    )
    r1 = ep_pool.tile([128, 256], f32, name="r1", tag="r1")
    nc.vector.tensor_add(r1[:, :], red8[:, 0:256], red8[:, 256:512])
    r2 = ep_pool.tile([128, 128], f32, name="r2", tag="r2")
    nc.vector.tensor_add(r2[:, :], r1[:, 0:128], r1[:, 128:256])
    dseT = ep_pool.tile([128, 64], f32, name="dseT", tag="dseT")
    nc.vector.tensor_add(dseT[:, :], r2[:, 0:64], r2[:, 64:128])

    if stage == "pack":
        nc.sync.dma_start(y_dbg[:, 0:64], dseT[:, :])
        nc.sync.dma_start(y_dbg[:, 64:65], gstats[:, :])
        nc.sync.dma_start(y_out[:], dseT[0, 0:64])
        for p in reversed(ctx_pools):
            p.__exit__(None, None, None)
        return

    # ---- exposed tail: normalize dse over D, final matmuls ----
    mean_d = ep_pool.tile([H1, 1], f32, name="mean_d", tag="mean_d")
    nc.vector.tensor_reduce(mean_d[:, :], dseT[:, :], axis=AX, op=ADD)
    nc.vector.tensor_scalar_mul(mean_d[:, :], mean_d[:, :], 1.0 / D)
    sqd = ep_pool.tile([H1, D], f32, name="sqd", tag="sqd")
    nc.scalar.activation(sqd[:, :], dseT[:, :], SQUARE)
    qd = ep_pool.tile([H1, 1], f32, name="qd", tag="qd")
    nc.vector.tensor_reduce(qd[:, :], sqd[:, :], axis=AX, op=ADD)
    vard = ep_pool.tile([H1, 1], f32, name="vard", tag="vard")
    nc.vector.tensor_mul(vard[:, :], mean_d[:, :], mean_d[:, :])
    nc.vector.tensor_scalar(vard[:, :], qd[:, :], 1.0 / D, vard[:, :],
                            op0=MUL, op1=SUB)
    stdd = ep_pool.tile([H1, 1], f32, name="stdd", tag="stdd")
    nc.scalar.activation(stdd[:, :], vard[:, :], SQRT)
    nc.vector.tensor_scalar_add(stdd[:, :], stdd[:, :], EPS)
    invd = ep_pool.tile([H1, 1], f32, name="invd", tag="invd")
    nc.vector.reciprocal(invd[:, :], stdd[:, :])
    dsen = ep_pool.tile([H1, D], f32, name="dsen", tag="dsen")
    nc.vector.tensor_scalar(dsen[:, :], dseT[:, :], mean_d[:, :], invd[:, :],
                            op0=SUB, op1=MUL)

    # h.T = leaky(W3 @ concat.T + b3); dsen chunk last (others pre-ready)
    psum_h = ep_psum.tile([H2, D], f32, name="psum_h", tag="ep")
    chunks = [dfeT[:, :], rl_b[:, :], repe[:, :], dsen[:, :]]
    order = [0, 2, 1, 3]
    for i, k in enumerate(order):
        nc.tensor.matmul(psum_h[:, :], lhsT=w3Tp[:, k * H2:(k + 1) * H2],
                         rhs=chunks[k], start=(i == 0), stop=(i == 3))
    hT = ep_pool.tile([H2, D], f32, name="hT", tag="hT")
    nc.scalar.activation(hT[:, :], psum_h[:, :], IDENT, bias=b3[:, :])
    hT_a = ep_pool.tile([H2, D], f32, name="hT_a", tag="hT_a")
    nc.vector.tensor_scalar_mul(hT_a[:, :], hT[:, :], SLOPE)
    nc.vector.tensor_max(hT[:, :], hT[:, :], hT_a[:, :])

    psum_o = ep_psum.tile([D, 1], f32, name="psum_o", tag="ep")
    nc.tensor.matmul(psum_o[:, :], lhsT=hT[:, :], rhs=w4T[:, :],
                     start=True, stop=True)
    out_sb = ep_pool.tile([D, 1], f32, name="out_sb", tag="out_sb")
    nc.scalar.activation(out_sb[:, :], psum_o[:, :], IDENT, bias=b4[:, :])
    nc.sync.dma_start(y_out[:], out_sb[:, 0])

    for p in reversed(ctx_pools):
        p.__exit__(None, None, None)


_compiled = None


def _get_compiled():
    global _compiled
    if _compiled is None:
        _compiled = build_program()
    return _compiled


def make_in_maps(inputs):
    state = np.asarray(inputs["state"], dtype=np.float32)
    dfs = np.asarray(inputs["device_feat_state"], dtype=np.float32)
    mpnn = np.asarray(inputs["mpnn_forward"], dtype=np.float32)
    W1 = np.asarray(inputs["W1"], dtype=np.float32)
    b1 = np.asarray(inputs["b1"], dtype=np.float32)
    W2 = np.asarray(inputs["W2"], dtype=np.float32)
    b2 = np.asarray(inputs["b2"], dtype=np.float32)
    W3 = np.asarray(inputs["W3"], dtype=np.float32)
    b3 = np.asarray(inputs["b3"], dtype=np.float32)
    W4 = np.asarray(inputs["W4"], dtype=np.float32)
    b4 = np.asarray(inputs["b4"], dtype=np.float32)
    mask = np.asarray(inputs["device_assign_state"])
    pred = int(np.asarray(inputs["pred_node"]))

    # block layouts: element (p, b*W + j) = full[b*128 + p, j]
    mp_l = mpnn.astype(NP_BF16).reshape(N // 128, 128, H1).transpose(1, 0, 2)
    st_l = state.astype(NP_F8).reshape(N // 128, 128, F).transpose(1, 0, 2)
    # 0/1 mask -> exact fp8e4m3 via the 0x38 (==1.0) byte trick
    mk8 = ((mask == 1).astype(np.uint8) * np.uint8(0x38)).view(NP_F8)
    mk_l = np.ascontiguousarray(mk8.reshape(D, N // 128, 128).transpose(1, 2, 0))
    # mk_l[b, p, d] = mask[d, b*128+p]

    w3Tp = np.ascontiguousarray(
        W3.T.reshape(4, H1, H2).transpose(1, 0, 2).reshape(H1, 4 * H2))
    common = {
        "x_dfsT": np.ascontiguousarray(np.pad(dfs.T, ((0, 64 - DF), (0, 0)))),
        "x_w1T": np.ascontiguousarray(np.pad(W1.T, ((0, 64 - DF), (0, 0)))),
        "x_b1": np.ascontiguousarray(b1.reshape(H1, 1)),
        "x_w2T": np.ascontiguousarray(W2.T),
        "x_b2": np.ascontiguousarray(b2.reshape(H1, 1)),
        "x_w3Tp": w3Tp,
        "x_b3": np.ascontiguousarray(b3.reshape(H2, 1)),
        "x_w4T": np.ascontiguousarray(W4.T),
        "x_b4": np.ascontiguousarray(np.broadcast_to(b4.reshape(1, 1), (D, 1))),
        "x_spred": np.ascontiguousarray(state[pred].reshape(F, 1)),
        "x_mpred": np.ascontiguousarray(mpnn[pred].reshape(H1, 1)),
    }
    in_maps = []
    for c in range(NCORES):
        bs = slice(c * NBLK, (c + 1) * NBLK)
        in_maps.append({
            **common,
            "x_mpnn": np.ascontiguousarray(mp_l[:, bs, :]).reshape(128, NBLK * H1),
            "x_state": np.ascontiguousarray(st_l[:, bs, :]).reshape(128, NBLK * F),
            "x_mask": np.ascontiguousarray(
                mk_l[bs].transpose(1, 0, 2)).reshape(128, NBLK * D),
        })
    return in_maps


def kernel(**inputs) -> np.ndarray:
    nc = _get_compiled()
    in_maps = make_in_maps(inputs)
    res = run_bass_kernel_spmd(nc, in_maps, core_ids=list(range(NCORES)))
    return np.asarray(res.results[0]["y_out"], dtype=np.float32)
